# revision 2
# baseline (speedup 1.0000x reference)
"""Trainium2 Bass kernel for nn_HNL_90185723281715 (scatter_memory).

Computation (see reference):
  q = x @ W_q.T                     [B, H, D]
  q_hat = q / ||q||                 (L2 over D)
  m_hat = memories / ||memories||   (L2 over D)
  s = q_hat . m_hat                 [B, H, M]   (cosine scores, in [-1, 1])
  p = softmax(s)                    (T=1; max-subtraction skipped -- s bounded)
  out = (p @ m_hat) * sqrt(D)       [B, H*D]

Sharding: tensor-parallel over heads -- 2 heads per core, full batch on
every core; host gathers/transposes the per-core [128, B] outputs.

Key implementation choices:
  * All matmul operands are bf16 (tolerance is 2e-2; measured rel err
    ~3e-3). fp32 accumulation in PSUM throughout.
  * Scores for the two heads are row-packed concurrent matmuls (K=64
    each, PE array rows 0-63 / 64-127 via tile_position).
  * The softmax denominator comes free from the combine matmul via a
    1/sqrt(D) ones-column appended to the combine weights, so
    acc[D] = Z/sqrt(D) and out = acc[0:D] * (1/acc[D]).
  * The `repeat` parameter wraps the body in a hardware For_i loop so
    benchmark NEFFs stay small. (An unrolled repeat loop makes the
    NEFF's per-run load/transfer cost scale with R, which swamps the
    actual ~0.5 ms/iteration HW execution time by 300x.)
"""

import numpy as np
from contextlib import ExitStack, nullcontext

import ml_dtypes

import concourse.bacc as bacc
import concourse.tile as tile
from concourse import mybir
from concourse.bass_utils import run_bass_kernel_spmd
from concourse.masks import make_identity

F32 = mybir.dt.float32
BF16 = mybir.dt.bfloat16

B = 4096          # batch rows
IN = 1024         # in features
H = 16            # heads total
M = 4096          # memories per head
D = 64            # head dim
N_CORES = 8
HPC = H // N_CORES    # 2 heads per core
G = 512               # batch rows per group
NG = B // G           # 8 row groups
NCH = M // 128        # 32 memory chunks of 128
RSQD = 0.125          # 1/sqrt(D): ones-column value so acc[D] = Z/sqrt(D)


def emit(tc, ctx, xT, wqT, mem, outT, blkA_in, blkB_in, repeat):
    nc = tc.nc
    ctx.enter_context(
        nc.allow_low_precision(reason="bf16 matmul operands")
    )

    const = ctx.enter_context(tc.tile_pool(name="const", bufs=1))
    persist = ctx.enter_context(tc.tile_pool(name="persist", bufs=1))
    expool = ctx.enter_context(tc.tile_pool(name="exp", bufs=3))
    small = ctx.enter_context(tc.tile_pool(name="small", bufs=1))
    small2 = ctx.enter_context(tc.tile_pool(name="small2", bufs=2))
    qnp = ctx.enter_context(tc.tile_pool(name="qn", bufs=2))

    # --- constants (outside the repeat loop) ---
    ident = const.tile([128, 128], BF16)
    make_identity(nc, ident[:])
    blkT = const.tile([2, 128], BF16)
    nc.sync.dma_start(blkT[:], blkB_in)
    blkones = const.tile([128, 2], BF16)
    nc.sync.dma_start(blkones[:], blkA_in)
    # W_q slice for this core's two heads, pre-transposed: wqT [IN, 128]
    wq = const.tile([128, 8, 128], BF16)   # [k-partition, k-chunk, feat]
    nc.sync.dma_start(wq[:], wqT.rearrange("(k p) f -> p k f", p=128))

    loop_ctx = tc.For_i(0, repeat, 1) if repeat > 1 else nullcontext()
    with loop_ctx:
        # =========== memories: load, normalize, append 1/sqrt(D) col =======
        memn1 = []
        for h in range(HPC):
            mt = persist.tile([128, NCH, D + 1], BF16, tag=f"memn1_{h}")
            nc.sync.dma_start(
                mt[:, :, 0:D], mem[h].rearrange("(c p) d -> p c d", p=128)
            )
            nc.vector.memset(mt[:, :, D], RSQD)
            memn1.append(mt)

        for h in range(HPC):
            sqt = small.tile([128, NCH, D], F32, tag="msq")
            nc.scalar.square(sqt[:], memn1[h][:, :, 0:D])
            ssq = small.tile([128, NCH], F32, tag="mssq")
            nc.vector.reduce_sum(ssq[:], sqt[:], axis=mybir.AxisListType.X)
            mnorm = small.tile([128, NCH], F32, tag="mnorm")
            nc.scalar.sqrt(mnorm[:], ssq[:])
            minv = small.tile([128, NCH], F32, tag="minv")
            nc.vector.reciprocal(minv[:], mnorm[:])
            for c in range(NCH):
                nc.vector.tensor_scalar_mul(
                    memn1[h][:, c, 0:D], memn1[h][:, c, 0:D], minv[:, c : c + 1]
                )

        # =========== memT: [128 (2 heads x 64 dims), 4096 mems] ============
        # PE-transpose normalized [128 mems, 64] chunks -> [64, 128 mems],
        # head h landing on partitions 64h..64h+63.
        memT = persist.tile([128, M], BF16, tag="memT")
        with tc.tile_pool(name="ptr", bufs=2, space="PSUM") as ptrp:
            for c4 in range(NCH // 4):
                pt = ptrp.tile([128, 512], BF16, tag="ptr")
                for h in range(HPC):
                    for j in range(4):
                        c = c4 * 4 + j
                        nc.tensor.transpose(
                            pt[h * D : (h + 1) * D, j * 128 : (j + 1) * 128],
                            memn1[h][:, c, 0:D],
                            ident[:],
                        )
                nc.vector.tensor_copy(
                    memT[:, c4 * 512 : (c4 + 1) * 512], pt[:]
                )

        # =========== q projection + normalize: qT [128, B] bf16 ============
        xk = []
        for k in range(8):
            xt = persist.tile([128, B], BF16, tag=f"xk{k}", name=f"xk{k}")
            nc.sync.dma_start(xt[:], xT[k * 128 : (k + 1) * 128, :])
            xk.append(xt)

        qT = persist.tile([128, B], BF16, tag="qT")
        with (
            tc.tile_pool(name="pq", bufs=2, space="PSUM") as pqp,
            tc.tile_pool(name="qmisc", bufs=2, space="PSUM") as qmp,
        ):
            for g in range(NG):
                gs = slice(g * G, (g + 1) * G)
                pq = pqp.tile([128, G], F32, tag="pq", name="pq")
                for k in range(8):
                    nc.tensor.matmul(
                        pq[:], wq[:, k, :], xk[k][:, gs],
                        start=(k == 0), stop=(k == 7),
                    )
                # per-(head, row) inverse norms via a K=128 head-selector
                # matmul over the squares, then sqrt + reciprocal.
                qsq = qnp.tile([128, G], BF16, tag="qsq")
                nc.scalar.square(qsq[:], pq[:])
                pns = qmp.tile([2, G], F32, tag="pns")
                nc.tensor.matmul(pns[:], blkones[:], qsq[:],
                                 start=True, stop=True)
                qn = qnp.tile([2, G], F32, tag="qn")
                nc.scalar.sqrt(qn[:], pns[:])
                qinv = qnp.tile([2, G], BF16, tag="qinv")
                nc.vector.reciprocal(qinv[:], qn[:])
                # broadcast the two rows onto the head halves (K=2 matmul)
                qbcp = qmp.tile([128, G], F32, tag="qbc")
                nc.tensor.matmul(qbcp[:], blkT[:], qinv[:],
                                 start=True, stop=True)
                qbcs = qnp.tile([128, G], F32, tag="qbcs")
                nc.vector.tensor_copy(qbcs[:], qbcp[:])
                nc.vector.tensor_mul(qT[:, gs], pq[:], qbcs[:])

        # =========== main loop: scores -> exp -> combine ===================
        with (
            tc.tile_pool(name="sc", bufs=3, space="PSUM") as scp,
            tc.tile_pool(name="acc", bufs=2, space="PSUM") as accp,
        ):
            for g in range(NG):
                gs = slice(g * G, (g + 1) * G)
                accs = [accp.tile([D + 1, G], F32, tag="acc", name=f"acc{h}")
                        for h in range(HPC)]
                for cp in range(NCH // 2):
                    scs = [scp.tile([128, 1024], F32, tag="sc", name="sc")
                           for _ in range(HPC)]
                    for i in range(2):
                        c = cp * 2 + i
                        for h in range(HPC):
                            nc.tensor.matmul(
                                scs[h][:, i * G : (i + 1) * G],
                                memT[h * D : (h + 1) * D,
                                     c * 128 : (c + 1) * 128],
                                qT[h * D : (h + 1) * D, gs],
                                start=True, stop=True,
                                tile_position=(h * D, 0),
                            )
                    exs = []
                    for h in range(HPC):
                        ex = expool.tile([128, 1024], BF16, tag="exp",
                                         name="ex")
                        nc.scalar.activation(
                            ex[:], scs[h][:],
                            mybir.ActivationFunctionType.Exp)
                        exs.append(ex)
                    for h in range(HPC):
                        for i in range(2):
                            c = cp * 2 + i
                            nc.tensor.matmul(
                                accs[h][:],
                                memn1[h][:, c, :],
                                exs[h][:, i * G : (i + 1) * G],
                                start=(c == 0), stop=(c == NCH - 1),
                            )
                # finalize: out = acc[0:D] * (sqrt(D)/Z); the 1/sqrt(D)
                # ones column makes acc[D] = Z/sqrt(D).
                for h in range(HPC):
                    dinv = small2.tile([1, G], F32, tag="dinv")
                    nc.vector.reciprocal(dinv[:], accs[h][D : D + 1, :])
                    bc = small2.tile([D, G], F32, tag="bc")
                    nc.gpsimd.partition_broadcast(bc[:], dinv[:])
                    ostage = small2.tile([D, G], F32, tag="ostage")
                    nc.vector.tensor_mul(ostage[:], accs[h][0:D, :], bc[:])
                    nc.sync.dma_start(outT[h * D : (h + 1) * D, gs],
                                      ostage[:])


def build(repeat=1):
    nc = bacc.Bacc(
        "TRN2", target_bir_lowering=False, debug=False, num_devices=N_CORES
    )
    xT_ap = nc.dram_tensor("xT", [IN, B], BF16, kind="ExternalInput").ap()
    wqT_ap = nc.dram_tensor("wqT", [IN, 128], BF16, kind="ExternalInput").ap()
    mem_ap = nc.dram_tensor("mem", [HPC, M, D], BF16, kind="ExternalInput").ap()
    outT_ap = nc.dram_tensor("outT", [128, B], F32, kind="ExternalOutput").ap()
    blkA_ap = nc.dram_tensor("blkA", [128, 2], BF16, kind="ExternalInput").ap()
    blkB_ap = nc.dram_tensor("blkB", [2, 128], BF16, kind="ExternalInput").ap()
    with tile.TileContext(nc) as tc, ExitStack() as ctx:
        emit(tc, ctx, xT_ap, wqT_ap, mem_ap, outT_ap, blkA_ap, blkB_ap, repeat)
    nc.compile()
    return nc


BLK_A = np.zeros((128, 2), ml_dtypes.bfloat16)
BLK_A[0:64, 0] = 1.0
BLK_A[64:128, 1] = 1.0
BLK_B = np.ascontiguousarray(BLK_A.T)


def run(x, W_q, memories, repeat=1, nc=None):
    if nc is None:
        nc = build(repeat)
    bf = ml_dtypes.bfloat16
    xT = np.ascontiguousarray(np.asarray(x).T.astype(bf))
    W_q = np.asarray(W_q)
    memories = np.asarray(memories)
    in_maps = []
    for i in range(N_CORES):
        in_maps.append(
            {
                "xT": xT,
                "wqT": np.ascontiguousarray(
                    W_q[i * 128 : (i + 1) * 128, :].T.astype(bf)
                ),
                "mem": np.ascontiguousarray(
                    memories[i * HPC : (i + 1) * HPC].astype(bf)
                ),
                "blkA": BLK_A,
                "blkB": BLK_B,
            }
        )
    res = run_bass_kernel_spmd(nc, in_maps, list(range(N_CORES)))
    out = np.empty((B, H * D), dtype=np.float32)
    for i in range(N_CORES):
        out[:, i * 128 : (i + 1) * 128] = res.results[i]["outT"].T
    return out


def kernel(x, W_q, memories):
    return run(x, W_q, memories)


# revision 3
# speedup vs baseline: 1.2410x; 1.2410x over previous
"""Trainium2 Bass kernel for nn_HNL_90185723281715 (scatter_memory).

Computation (see reference):
  q = x @ W_q.T                     [B, H, D]
  q_hat = q / ||q||                 (L2 over D)
  m_hat = memories / ||memories||   (L2 over D)
  s = q_hat . m_hat                 [B, H, M]   (cosine scores, in [-1, 1])
  p = softmax(s)                    (T=1; max-subtraction skipped -- s bounded)
  out = (p @ m_hat) * sqrt(D)       [B, H*D]

Sharding: tensor-parallel over heads -- 2 heads per core, full batch on
every core; host gathers/transposes the per-core [128, B] outputs.

Key implementation choices:
  * All matmul operands are bf16 (tolerance is 2e-2; measured rel err
    ~3e-3). fp32 accumulation in PSUM throughout.
  * Scores for the two heads are row-packed concurrent matmuls (K=64
    each, PE array rows 0-63 / 64-127 via tile_position).
  * The softmax denominator comes free from the combine matmul via a
    1/sqrt(D) ones-column appended to the combine weights, so
    acc[D] = Z/sqrt(D) and out = acc[0:D] * (1/acc[D]).
  * The `repeat` parameter wraps the body in a hardware For_i loop so
    benchmark NEFFs stay small. (An unrolled repeat loop makes the
    NEFF's per-run load/transfer cost scale with R, which swamps the
    actual ~0.5 ms/iteration HW execution time by 300x.)
"""

import numpy as np
from contextlib import ExitStack, nullcontext

import ml_dtypes

import concourse.bacc as bacc
import concourse.tile as tile
from concourse import mybir
from concourse.bass_utils import run_bass_kernel_spmd
from concourse.masks import make_identity

F32 = mybir.dt.float32
BF16 = mybir.dt.bfloat16

B = 4096          # batch rows
IN = 1024         # in features
H = 16            # heads total
M = 4096          # memories per head
D = 64            # head dim
N_CORES = 8
HPC = H // N_CORES    # 2 heads per core
G = 512               # batch rows per group
NG = B // G           # 8 row groups
NCH = M // 128        # 32 memory chunks of 128
LAG = 2               # combine lag (chunk-pairs) in the software pipeline
RSQD = 0.125          # 1/sqrt(D): ones-column value so acc[D] = Z/sqrt(D)


def emit(tc, ctx, xT, wqT, mem, outT, blkA_in, blkB_in, repeat):
    nc = tc.nc
    ctx.enter_context(
        nc.allow_low_precision(reason="bf16 matmul operands")
    )

    const = ctx.enter_context(tc.tile_pool(name="const", bufs=1))
    persist = ctx.enter_context(tc.tile_pool(name="persist", bufs=1))
    expool = ctx.enter_context(tc.tile_pool(name="exp", bufs=2 * (LAG + 2)))
    small = ctx.enter_context(tc.tile_pool(name="small", bufs=1))
    small2 = ctx.enter_context(tc.tile_pool(name="small2", bufs=2))
    qnp = ctx.enter_context(tc.tile_pool(name="qn", bufs=2))

    # --- constants (outside the repeat loop) ---
    ident = const.tile([128, 128], BF16)
    make_identity(nc, ident[:])
    blkT = const.tile([2, 128], BF16)
    nc.sync.dma_start(blkT[:], blkB_in)
    blkones = const.tile([128, 2], BF16)
    nc.sync.dma_start(blkones[:], blkA_in)
    # W_q slice for this core's two heads, pre-transposed: wqT [IN, 128]
    wq = const.tile([128, 8, 128], BF16)   # [k-partition, k-chunk, feat]
    nc.sync.dma_start(wq[:], wqT.rearrange("(k p) f -> p k f", p=128))

    loop_ctx = tc.For_i(0, repeat, 1) if repeat > 1 else nullcontext()
    with loop_ctx:
        # =========== memories: load, normalize, append 1/sqrt(D) col =======
        memn1 = []
        for h in range(HPC):
            mt = persist.tile([128, NCH, D + 1], BF16, tag=f"memn1_{h}")
            nc.sync.dma_start(
                mt[:, :, 0:D], mem[h].rearrange("(p c) d -> p c d", c=NCH)
            )
            nc.vector.memset(mt[:, :, D], RSQD)
            memn1.append(mt)

        for h in range(HPC):
            sqt = small.tile([128, NCH, D], F32, tag="msq")
            nc.scalar.square(sqt[:], memn1[h][:, :, 0:D])
            ssq = small.tile([128, NCH], F32, tag="mssq")
            nc.vector.reduce_sum(ssq[:], sqt[:], axis=mybir.AxisListType.X)
            mnorm = small.tile([128, NCH], F32, tag="mnorm")
            nc.scalar.sqrt(mnorm[:], ssq[:])
            minv = small.tile([128, NCH], F32, tag="minv")
            nc.vector.reciprocal(minv[:], mnorm[:])
            for c in range(NCH):
                nc.vector.tensor_scalar_mul(
                    memn1[h][:, c, 0:D], memn1[h][:, c, 0:D], minv[:, c : c + 1]
                )

        # =========== memT: [128 (2 heads x 64 dims), 4096 mems] ============
        # PE-transpose normalized [128 mems, 64] chunks -> [64, 128 mems],
        # head h landing on partitions 64h..64h+63.
        memT = persist.tile([128, M], BF16, tag="memT")
        with tc.tile_pool(name="ptr", bufs=2, space="PSUM") as ptrp:
            for c4 in range(NCH // 4):
                pt = ptrp.tile([128, 512], BF16, tag="ptr")
                for h in range(HPC):
                    for j in range(4):
                        c = c4 * 4 + j
                        nc.tensor.transpose(
                            pt[h * D : (h + 1) * D, j * 128 : (j + 1) * 128],
                            memn1[h][:, c, 0:D],
                            ident[:],
                        )
                nc.vector.tensor_copy(
                    memT[:, c4 * 512 : (c4 + 1) * 512], pt[:]
                )

        # =========== q projection + normalize: qT [128, B] bf16 ============
        xk = []
        for k in range(8):
            xt = persist.tile([128, B], BF16, tag=f"xk{k}", name=f"xk{k}")
            nc.sync.dma_start(xt[:], xT[k * 128 : (k + 1) * 128, :])
            xk.append(xt)

        qT = persist.tile([128, B], BF16, tag="qT")
        with (
            tc.tile_pool(name="pq", bufs=2, space="PSUM") as pqp,
            tc.tile_pool(name="qmisc", bufs=2, space="PSUM") as qmp,
        ):
            for g in range(NG):
                gs = slice(g * G, (g + 1) * G)
                pq = pqp.tile([128, G], F32, tag="pq", name="pq")
                for k in range(8):
                    nc.tensor.matmul(
                        pq[:], wq[:, k, :], xk[k][:, gs],
                        start=(k == 0), stop=(k == 7),
                    )
                # per-(head, row) inverse norms via a K=128 head-selector
                # matmul over the squares, then sqrt + reciprocal.
                qsq = qnp.tile([128, G], BF16, tag="qsq")
                nc.scalar.square(qsq[:], pq[:])
                pns = qmp.tile([2, G], F32, tag="pns")
                nc.tensor.matmul(pns[:], blkones[:], qsq[:],
                                 start=True, stop=True)
                qn = qnp.tile([2, G], F32, tag="qn")
                nc.scalar.sqrt(qn[:], pns[:])
                qinv = qnp.tile([2, G], BF16, tag="qinv")
                nc.vector.reciprocal(qinv[:], qn[:])
                # broadcast the two rows onto the head halves (K=2 matmul)
                qbcp = qmp.tile([128, G], F32, tag="qbc")
                nc.tensor.matmul(qbcp[:], blkT[:], qinv[:],
                                 start=True, stop=True)
                qbcs = qnp.tile([128, G], F32, tag="qbcs")
                nc.vector.tensor_copy(qbcs[:], qbcp[:])
                nc.vector.tensor_mul(qT[:, gs], pq[:], qbcs[:])

        # =========== main loop: scores -> exp -> combine ===================
        with (
            tc.tile_pool(name="sc", bufs=3, space="PSUM") as scp,
            tc.tile_pool(name="acc", bufs=2, space="PSUM") as accp,
        ):
            for g in range(NG):
                gs = slice(g * G, (g + 1) * G)
                accs = [accp.tile([D + 1, G], F32, tag="acc", name=f"acc{h}")
                        for h in range(HPC)]
                # scs[i] covers chunk cp*2+i; the free-dim halves hold
                # the two heads, so the row-packed score-MM pair shares one
                # tile allocation and stays adjacent in the PE queue
                # (required for concurrent row-group execution). Combines
                # lag scores by LAG chunk-pairs so the in-order PE queue
                # never stalls waiting on an exp.
                exq = []

                def emit_scores(cp):
                    scs = [scp.tile([128, 1024], F32, tag="sc", name="sc")
                           for _ in range(2)]
                    for i in range(2):
                        c = cp * 2 + i
                        for h in range(HPC):
                            nc.tensor.matmul(
                                scs[i][:, h * G : (h + 1) * G],
                                memT[h * D : (h + 1) * D,
                                     c * 128 : (c + 1) * 128],
                                qT[h * D : (h + 1) * D, gs],
                                start=True, stop=True,
                                tile_position=(h * D, 0),
                            )
                    exs = []
                    for i in range(2):
                        ex = expool.tile([128, 1024], BF16, tag="exp",
                                         name="ex")
                        nc.scalar.activation(
                            ex[:], scs[i][:],
                            mybir.ActivationFunctionType.Exp)
                        exs.append(ex)
                    exq.append(exs)

                def emit_combines(cp):
                    exs = exq[cp]
                    for h in range(HPC):
                        for i in range(2):
                            c = cp * 2 + i
                            nc.tensor.matmul(
                                accs[h][:],
                                memn1[h][:, c, :],
                                exs[i][:, h * G : (h + 1) * G],
                                start=(c == 0), stop=(c == NCH - 1),
                            )
                    exq[cp] = None

                for cp in range(NCH // 2):
                    emit_scores(cp)
                    if cp >= LAG:
                        emit_combines(cp - LAG)
                for cp in range(NCH // 2 - LAG, NCH // 2):
                    emit_combines(cp)
                # finalize: out = acc[0:D] * (sqrt(D)/Z); the 1/sqrt(D)
                # ones column makes acc[D] = Z/sqrt(D).
                for h in range(HPC):
                    dinv = small2.tile([1, G], F32, tag="dinv")
                    nc.vector.reciprocal(dinv[:], accs[h][D : D + 1, :])
                    bc = small2.tile([D, G], F32, tag="bc")
                    nc.gpsimd.partition_broadcast(bc[:], dinv[:])
                    ostage = small2.tile([D, G], F32, tag="ostage")
                    nc.vector.tensor_mul(ostage[:], accs[h][0:D, :], bc[:])
                    nc.sync.dma_start(outT[h * D : (h + 1) * D, gs],
                                      ostage[:])


def build(repeat=1):
    nc = bacc.Bacc(
        "TRN2", target_bir_lowering=False, debug=False, num_devices=N_CORES
    )
    xT_ap = nc.dram_tensor("xT", [IN, B], BF16, kind="ExternalInput").ap()
    wqT_ap = nc.dram_tensor("wqT", [IN, 128], BF16, kind="ExternalInput").ap()
    mem_ap = nc.dram_tensor("mem", [HPC, M, D], BF16, kind="ExternalInput").ap()
    outT_ap = nc.dram_tensor("outT", [128, B], F32, kind="ExternalOutput").ap()
    blkA_ap = nc.dram_tensor("blkA", [128, 2], BF16, kind="ExternalInput").ap()
    blkB_ap = nc.dram_tensor("blkB", [2, 128], BF16, kind="ExternalInput").ap()
    with tile.TileContext(nc) as tc, ExitStack() as ctx:
        emit(tc, ctx, xT_ap, wqT_ap, mem_ap, outT_ap, blkA_ap, blkB_ap, repeat)
    nc.compile()
    return nc


BLK_A = np.zeros((128, 2), ml_dtypes.bfloat16)
BLK_A[0:64, 0] = 1.0
BLK_A[64:128, 1] = 1.0
BLK_B = np.ascontiguousarray(BLK_A.T)


def run(x, W_q, memories, repeat=1, nc=None):
    if nc is None:
        nc = build(repeat)
    bf = ml_dtypes.bfloat16
    xT = np.ascontiguousarray(np.asarray(x).T.astype(bf))
    W_q = np.asarray(W_q)
    memories = np.asarray(memories)
    in_maps = []
    for i in range(N_CORES):
        in_maps.append(
            {
                "xT": xT,
                "wqT": np.ascontiguousarray(
                    W_q[i * 128 : (i + 1) * 128, :].T.astype(bf)
                ),
                "mem": np.ascontiguousarray(
                    memories[i * HPC : (i + 1) * HPC].astype(bf)
                ),
                "blkA": BLK_A,
                "blkB": BLK_B,
            }
        )
    res = run_bass_kernel_spmd(nc, in_maps, list(range(N_CORES)))
    out = np.empty((B, H * D), dtype=np.float32)
    for i in range(N_CORES):
        out[:, i * 128 : (i + 1) * 128] = res.results[i]["outT"].T
    return out


def kernel(x, W_q, memories):
    return run(x, W_q, memories)
